# revision 27
# baseline (speedup 1.0000x reference)
"""Trainium2 Bass kernel for nn_FHNet (batch[64,2048,784] @ W1.T -> FHN scan
-> *0.5 @ W2.T -> FHN scan), data-parallel over batch across 8 NeuronCores.

Per core (8 samples):
- mm1 on PE: K=784 in 7 chunks, batch pre-transposed host-side to
  [8, 784, 2048] so the contraction dim lands on partitions. Weights
  pre-scaled host-side so all constant folds (dt, 0.5, k/beta rescale,
  gamma shift) are free.
- FHN scans as per-step stock DVE ops on [100, 8] (scan1: feature on
  partitions, samples on free) / [80, 1] (scan2) column slices.
  Rescaled recurrence (Vt = k*V, Z = (k/beta)*(q - W)):
      Vt' = Vt*(A - Vt^2) + beta*Z ;  Z' = alpha*Z - Vt + p_t
  with p precomputed in bulk from the matmul outputs.
- mm2 on PE (K=100, single matmul per 512-token chunk).
- Output via strided DMA ([10, T] SBUF -> [T, 10] DRAM).

This container's walrus accepts at most ONE sync wait per engine
instruction; Tile emits more. `_split_multi_waits` hoists extras into
preceding same-engine EventSemaphore instructions (in-order execution
keeps semantics identical).
"""
import json
import sys
import numpy as np

sys.path.insert(0, "/opt/trn_rl_repo")

# ---------------- constants ----------------
DT = 0.04
A_CONST = float(1.0 + DT)
ALPHA = float(1.0 - DT * 0.08 * 0.8)
BETA = float(DT * DT * 0.08)
GAMMA = float(DT * DT * 0.08 * 0.7)
K_SC = float(np.sqrt(DT / 3.0))
KOB = float(K_SC / BETA)
C_BIAS = float(KOB * GAMMA / (1.0 - ALPHA))

B, T, D, N, M = 64, 2048, 784, 100, 10
NCORES = 8
BL = B // NCORES
KC = 7
KCH = D // KC          # 112
TCH = 512
NTC = T // TCH
TP = T + 1

_CACHE = {}


# ------------- walrus single-wait workaround -------------
def _split_multi_waits(bir_json_bytes: bytes) -> bytes:
    d = json.loads(bir_json_bytes)
    for fn in d.get("functions", []):
        for blk in fn.get("blocks", []):
            out = []
            for inst in blk.get("instructions", []):
                si = inst.get("sync_info")
                waits = (si or {}).get("on_wait") or []
                if len(waits) > 1:
                    for k, w in enumerate(waits[:-1]):
                        ev = {
                            "engine": inst["engine"],
                            "ins": [],
                            "outs": [],
                            "name": f"{inst['name']}_hw{k}",
                            "opcode": "EventSemaphore",
                            "sync_info": {"on_update": [], "on_wait": [w]},
                        }
                        if "debug" in inst:
                            ev["debug"] = inst["debug"]
                        out.append(ev)
                    si["on_wait"] = waits[-1:]
                out.append(inst)
            blk["instructions"] = out
    return json.dumps(d).encode()


def _install_bir_patch():
    import concourse.bass_utils as bu
    import concourse.bass2jax as b2j

    if getattr(bu, "_multiwait_patched", False):
        return
    orig = bu.compile_bir_kernel

    def patched(bir_json, tmpdir, neff_name="file.neff"):
        if isinstance(bir_json, str):
            bir_json = bir_json.encode()
        return orig(_split_multi_waits(bir_json), tmpdir, neff_name=neff_name)

    bu.compile_bir_kernel = patched
    bu._multiwait_patched = True
    b2j.compile_bir_kernel = patched


def _register_fhn_ops():
    """Register the two fused FHN-step custom DVE ops (documented extension
    point: dve_ops.OPS + _SUB_OPCODE_FOR_NAME + CUSTOM_DVE_SPECS).

    Reformulated recurrence (scaled vars Vt, H; G := Vt*(A - Vt^2)):
        Vt[t+2] = G(Vt[t+1]) + alpha*Vt[t+1] - H[t]          (STEP_V)
        H[t+1]  = alpha*G(Vt[t+1]) + beta*(Vt[t+1] - p[t+1]) (STEP_H)
    equivalent to the baseline (Vt, Z) system with H[t] =
    alpha*G(Vt[t]) + beta*Vt[t] - beta*p[t]; bootstrap:
        Vt[1] = beta*p1col0,  H[0] = -beta*p1col1.
    """
    import concourse.dve_ops as dops
    from concourse.dve_spec import Spec, Src0, Src1, C0, C1, C2, sq, lower, _has_src1
    from concourse.dve_uop import DveOpSpec

    if "FHN_STEP_V_ANT" in dops._SUB_OPCODE_FOR_NAME:
        return

    defs = [
        ("FHN_STEP_V_ANT",
         (C0 - sq(Src0)) * Src0 + C1 * Src0 - Src1,
         lambda in0, in1, s0, s1, imm2:
             (np.float32(s0) - in0 * in0) * in0 + np.float32(s1) * in0 - in1),
        ("FHN_STEP_H_ANT",
         C0 * ((C2 - sq(Src0)) * Src0) + C1 * (Src0 - Src1),
         lambda in0, in1, s0, s1, imm2:
             np.float32(s0) * ((np.float32(imm2) - in0 * in0) * in0)
             + np.float32(s1) * (in0 - in1)),
    ]
    for name, body, ref in defs:
        row = max(dops._SUB_OPCODE_FOR_NAME.values()) + 1
        assert row < 0x20
        spec = Spec(body=body, reference=ref)
        shas = {}
        for ver in ("v3", "v4"):
            uops = lower(spec, ver=ver)
            shas[ver] = DveOpSpec(
                name=name, opcode=row, uops=uops, rd1_en=_has_src1(spec)
            ).sha(ver)
        op = dops.DveOp(name, spec, subdim=False, uops_sha=shas)
        dops._SUB_OPCODE_FOR_NAME[name] = row
        dops.OPS.append(op)
        dops.CUSTOM_DVE_SPECS[name] = spec


def _build_kernel_v2(scan_steps=None):
    """Same structure as v1 but the FHN scans run 2 fused custom-DVE
    instructions per time step (vs 5 ops incl. a ScalarE round-trip)."""
    _register_fhn_ops()
    import concourse.bass as bass
    import concourse.tile as tile
    from concourse import mybir
    from concourse.dve_ops import CUSTOM_DVE_SPECS, OPS

    step_v = next(o for o in OPS if o.name == "FHN_STEP_V_ANT")
    step_h = next(o for o in OPS if o.name == "FHN_STEP_H_ANT")

    f32 = mybir.dt.float32
    AOp = mybir.AluOpType

    nc = bass.Bass()
    _cb = nc.alloc_sbuf_tensor("const-cbias", [128, 1], f32)
    nc.gpsimd.memset(_cb.ap(), -C_BIAS)
    nc.const_aps.aps[(f32, -C_BIAS)] = _cb.ap()
    nc.all_engine_barrier()

    bt_d = nc.declare_dram_parameter("batchT", [BL, D, T], f32, isOutput=False)
    w1_d = nc.declare_dram_parameter("W1T", [D, N], f32, isOutput=False)
    w2_d = nc.declare_dram_parameter("W2T", [N, M], f32, isOutput=False)
    out_d = nc.declare_dram_parameter("out", [BL, T, M], f32, isOutput=True)

    with tile.TileContext(nc) as tc:
        with (
            tc.tile_pool(name="const", bufs=1) as cpool,
            tc.tile_pool(name="bt", bufs=3) as btpool,
            tc.tile_pool(name="qs", bufs=3) as qspool,
            tc.tile_pool(name="ps1", bufs=4, space="PSUM") as ps1pool,
            tc.tile_pool(name="ps2", bufs=2, space="PSUM") as ps2pool,
            tc.tile_pool(name="big", bufs=1) as bigpool,
            tc.tile_pool(name="small", bufs=3) as spool,
            tc.tile_pool(name="state", bufs=2) as stpool,
        ):
            w1t = cpool.tile([KCH, KC * N], f32)
            for i in range(KC):
                nc.sync.dma_start(
                    w1t[:, i * N:(i + 1) * N], w1_d[i * KCH:(i + 1) * KCH, :]
                )
            w2t = cpool.tile([N, M], f32)
            nc.sync.dma_start(w2t[:], w2_d[:])

            p1 = bigpool.tile([N, BL * TP], f32)
            v1 = bigpool.tile([N, BL * T], f32)
            q2 = bigpool.tile([BL * M, T], f32)
            p2 = bigpool.tile([BL * M, TP], f32)

            p1_3 = p1[:].rearrange("p (b t) -> p b t", b=BL)
            v1_3 = v1[:].rearrange("p (b t) -> p b t", b=BL)

            # ---------------- mm1 + per-sample p-stream build ---------------
            for b in range(BL):
                qb = qspool.tile([N, T], f32, tag="qhat")
                for c in range(NTC):
                    ps = ps1pool.tile([N, TCH], f32)
                    for i in range(KC):
                        bt = btpool.tile([KCH, TCH], f32)
                        nc.sync.dma_start(
                            bt[:],
                            bt_d[b, i * KCH:(i + 1) * KCH,
                                 c * TCH:(c + 1) * TCH],
                        )
                        nc.tensor.matmul(
                            ps[:], lhsT=w1t[:, i * N:(i + 1) * N], rhs=bt[:],
                            start=(i == 0), stop=(i == KC - 1),
                        )
                    nc.scalar.add(qb[:, c * TCH:(c + 1) * TCH], ps[:], -C_BIAS)
                s = p1[:, b * TP:(b + 1) * TP]
                nc.vector.tensor_scalar(
                    s[:, 0:1], qb[:, 0:1], C_BIAS, None, AOp.add)
                nc.vector.scalar_tensor_tensor(
                    s[:, 1:T], qb[:, 0:T - 1], -ALPHA, qb[:, 1:T],
                    AOp.mult, AOp.add)

            # ---------------- scan 1: 2 fused ops per step ------------------
            _S1 = (T - 2) if scan_steps is None else min(scan_steps, T - 2)
            nc.vector.memset(v1_3[:, :, 0], 0.0)
            nc.vector.tensor_scalar(v1_3[:, :, 1], p1_3[:, :, 0], BETA, None,
                                    AOp.mult)
            ha = stpool.tile([N, BL], f32, tag="ha")
            hb = stpool.tile([N, BL], f32, tag="hb")
            nc.vector.tensor_scalar(ha[:], p1_3[:, :, 1], -BETA, None,
                                    AOp.mult)
            for t in range(_S1):
                hs, hn = (ha, hb) if t % 2 == 0 else (hb, ha)
                nc.vector._custom_dve(
                    step_v, out=v1_3[:, :, t + 2], in0=v1_3[:, :, t + 1],
                    in1=hs[:], s0=A_CONST, s1=ALPHA)
                if t < _S1 - 1:
                    nc.vector._custom_dve(
                        step_h, out=hn[:], in0=v1_3[:, :, t + 1],
                        in1=p1_3[:, :, t + 2], s0=ALPHA, s1=BETA, imm2=A_CONST)

            # ---------------- mm2 ------------------------------------------
            for b in range(BL):
                for c in range(NTC):
                    ps2 = ps2pool.tile([M, TCH], f32)
                    nc.tensor.matmul(
                        ps2[:], lhsT=w2t[:],
                        rhs=v1[:, b * T + c * TCH: b * T + (c + 1) * TCH],
                        start=True, stop=True)
                    st2 = spool.tile([M, TCH], f32, tag="q2st")
                    nc.scalar.add(st2[:], ps2[:], -C_BIAS)
                    nc.sync.dma_start(
                        q2[b * M:(b + 1) * M, c * TCH:(c + 1) * TCH], st2[:])

            # ---------------- scan 2 ([80, 1] slices) -----------------------
            nc.vector.tensor_scalar(
                p2[:, 0:1], q2[:, 0:1], C_BIAS, None, AOp.add)
            nc.vector.scalar_tensor_tensor(
                p2[:, 1:T], q2[:, 0:T - 1], -ALPHA, q2[:, 1:T],
                AOp.mult, AOp.add)

            P2 = BL * M
            _S2 = (T - 2) if scan_steps is None else min(scan_steps, T - 2)
            nc.vector.memset(q2[:, 0:1], 0.0)
            nc.vector.tensor_scalar(q2[:, 1:2], p2[:, 0:1], BETA, None,
                                    AOp.mult)
            h2a = stpool.tile([P2, 1], f32, tag="h2a")
            h2b = stpool.tile([P2, 1], f32, tag="h2b")
            nc.vector.tensor_scalar(h2a[:], p2[:, 1:2], -BETA, None, AOp.mult)
            for t in range(_S2):
                hs, hn = (h2a, h2b) if t % 2 == 0 else (h2b, h2a)
                nc.vector._custom_dve(
                    step_v, out=q2[:, t + 2:t + 3], in0=q2[:, t + 1:t + 2],
                    in1=hs[:], s0=A_CONST, s1=ALPHA)
                if t < _S2 - 1:
                    nc.vector._custom_dve(
                        step_h, out=hn[:], in0=q2[:, t + 1:t + 2],
                        in1=p2[:, t + 2:t + 3], s0=ALPHA, s1=BETA,
                        imm2=A_CONST)

            # unscale into p2 (dead) and DMA out
            nc.vector.tensor_scalar(p2[:, 0:T], q2[:], 1.0 / K_SC, None,
                                    AOp.mult)
            for b in range(BL):
                nc.sync.dma_start(
                    out_d[b].rearrange("t m -> m t"),
                    p2[b * M:(b + 1) * M, 0:T])

    return nc


def _build_kernel_v3(scan_steps=None, reps=1):
    """Chunk-pipelined: mm1 chunk c+1 (PE/DMA) overlaps scan1 segment c
    (Act square + 4 DVE ops/step); mm2 chunk c (PE) runs between segments;
    scan2 segment c-1 (same 5-op pattern) interleaves instruction-by-
    instruction with scan1 segment c so the two chains fill each other's
    cross-engine latency. Scan2 writes V2 to a separate tile (q2 keeps
    qhat2 for the chunked p2 builds).

    reps > 1 repeats the whole computation (including all DMA) inside one
    NEFF — used by test.py to amortize per-dispatch overhead when timing;
    every rep recomputes the identical result from DRAM inputs."""
    assert scan_steps is None
    import concourse.bass as bass
    import concourse.tile as tile
    from concourse import mybir

    f32 = mybir.dt.float32
    AOp = mybir.AluOpType
    SQ = mybir.ActivationFunctionType.Square

    nc = bass.Bass()
    _cb = nc.alloc_sbuf_tensor("const-cbias", [128, 1], f32)
    nc.gpsimd.memset(_cb.ap(), -C_BIAS)
    nc.const_aps.aps[(f32, -C_BIAS)] = _cb.ap()
    nc.all_engine_barrier()

    bt_d = nc.declare_dram_parameter("batchT", [BL, D, T], f32, isOutput=False)
    w1_d = nc.declare_dram_parameter("W1T", [D, N], f32, isOutput=False)
    w2_d = nc.declare_dram_parameter("W2T", [N, M], f32, isOutput=False)
    out_d = nc.declare_dram_parameter("out", [BL, T, M], f32, isOutput=True)

    P2 = BL * M

    with tile.TileContext(nc) as tc:
        with (
            tc.tile_pool(name="const", bufs=1) as cpool,
            tc.tile_pool(name="bt", bufs=4) as btpool,
            tc.tile_pool(name="qs", bufs=8) as qspool,
            tc.tile_pool(name="ps1", bufs=4, space="PSUM") as ps1pool,
            tc.tile_pool(name="ps2", bufs=4, space="PSUM") as ps2pool,
            tc.tile_pool(name="big", bufs=1) as bigpool,
            tc.tile_pool(name="small", bufs=6) as spool,
            tc.tile_pool(name="st2p", bufs=4) as st2pool,
            tc.tile_pool(name="state", bufs=2) as stpool,
        ):
            w1t = cpool.tile([KCH, KC * N], f32)
            w2t = cpool.tile([N, M], f32)
            lastq = cpool.tile([N, BL], f32)

            p1 = bigpool.tile([N, BL * TP], f32)
            v1 = bigpool.tile([N, BL * T], f32)
            q2 = bigpool.tile([P2, T], f32)     # qhat2 (mm2 output)
            p2 = bigpool.tile([P2, TP], f32)    # scan-2 p-stream
            v2 = bigpool.tile([P2, T], f32)     # scan-2 output (Vt2)

            p1_3 = p1[:].rearrange("p (b t) -> p b t", b=BL)
            v1_3 = v1[:].rearrange("p (b t) -> p b t", b=BL)

            # ---------------- emission helpers -----------------------------
            qbs = {}

            def mm1_chunk_pe(c, defer_bias=True):
                """DMA + PE matmuls for chunk c; Act bias adds returned as
                closures (interleaved into the scan stream so the Act queue
                never stalls the scan squares behind an unready add)."""
                closures = []
                for b in range(BL):
                    ps = ps1pool.tile([N, TCH], f32)
                    for i in range(KC):
                        bt = btpool.tile([KCH, TCH], f32)
                        nc.sync.dma_start(
                            bt[:],
                            bt_d[b, i * KCH:(i + 1) * KCH,
                                 c * TCH:(c + 1) * TCH],
                        )
                        nc.tensor.matmul(
                            ps[:], lhsT=w1t[:, i * N:(i + 1) * N], rhs=bt[:],
                            start=(i == 0), stop=(i == KC - 1),
                        )

                    def bias(b=b, ps=ps):
                        qb = qspool.tile([N, TCH], f32, tag="qhat")
                        nc.scalar.add(qb[:], ps[:], -C_BIAS)
                        qbs[(b, c)] = qb
                    if defer_bias:
                        closures.append(bias)
                    else:
                        bias()
                return closures

            def p1_build(c):
                """DVE p1-stream build for chunk c (deferred past the scans
                emitted earlier in the phase so the DVE never waits on mm1)."""
                for b in range(BL):
                    qb = qbs.pop((b, c))
                    s = p1[:, b * TP:(b + 1) * TP]
                    t0 = c * TCH
                    if c == 0:
                        nc.vector.tensor_scalar(
                            s[:, 0:1], qb[:, 0:1], C_BIAS, None, AOp.add)
                    else:
                        nc.vector.scalar_tensor_tensor(
                            s[:, t0:t0 + 1], lastq[:, b:b + 1], -ALPHA,
                            qb[:, 0:1], AOp.mult, AOp.add)
                    nc.vector.scalar_tensor_tensor(
                        s[:, t0 + 1:t0 + TCH], qb[:, 0:TCH - 1], -ALPHA,
                        qb[:, 1:TCH], AOp.mult, AOp.add)
                    if c < NTC - 1:
                        nc.vector.tensor_copy(
                            lastq[:, b:b + 1], qb[:, TCH - 1:TCH])

            za = stpool.tile([N, BL], f32, tag="za")
            zb = stpool.tile([N, BL], f32, tag="zb")

            def scan1_step(t):
                # Emission order: square first (Act), then the Z-chain ops
                # (independent of the square) so the Act->DVE handoff has
                # two DVE ops of slack before m consumes u.
                zs, zn = (za, zb) if t % 2 == 0 else (zb, za)
                vt = v1_3[:, :, t]
                u = spool.tile([N, BL], f32, tag="u")
                nc.scalar.activation(u[:], vt, SQ)
                t2 = spool.tile([N, BL], f32, tag="t2")
                nc.vector.scalar_tensor_tensor(
                    t2[:], zs[:], ALPHA, vt, AOp.mult, AOp.subtract)
                nc.vector.tensor_tensor(zn[:], t2[:], p1_3[:, :, t + 1],
                                        AOp.add)
                m = spool.tile([N, BL], f32, tag="m")
                nc.vector.scalar_tensor_tensor(
                    m[:], u[:], A_CONST, vt, AOp.subtract, AOp.mult)
                nc.vector.scalar_tensor_tensor(
                    v1_3[:, :, t + 1], zs[:], BETA, m[:], AOp.mult,
                    AOp.subtract)

            za2 = stpool.tile([P2, 1], f32, tag="za2")
            zb2 = stpool.tile([P2, 1], f32, tag="zb2")

            def scan2_step(t):
                zs, zn = (za2, zb2) if t % 2 == 0 else (zb2, za2)
                vt = v2[:, t:t + 1]
                u = spool.tile([P2, 1], f32, tag="u2")
                nc.scalar.activation(u[:], vt, SQ)
                t2 = spool.tile([P2, 1], f32, tag="t22")
                nc.vector.scalar_tensor_tensor(
                    t2[:], zs[:], ALPHA, vt, AOp.mult, AOp.subtract)
                nc.vector.tensor_tensor(zn[:], t2[:], p2[:, t + 1:t + 2],
                                        AOp.add)
                m = spool.tile([P2, 1], f32, tag="m2")
                nc.vector.scalar_tensor_tensor(
                    m[:], u[:], A_CONST, vt, AOp.subtract, AOp.mult)
                nc.vector.scalar_tensor_tensor(
                    v2[:, t + 1:t + 2], zs[:], BETA, m[:], AOp.mult,
                    AOp.subtract)

            def mm2_mm(c):
                """PE matmuls for mm2 chunk c; returns Act/DMA tail closures."""
                closures = []
                for b in range(BL):
                    ps2 = ps2pool.tile([M, TCH], f32)
                    nc.tensor.matmul(
                        ps2[:], lhsT=w2t[:],
                        rhs=v1[:, b * T + c * TCH: b * T + (c + 1) * TCH],
                        start=True, stop=True)

                    def tail(b=b, ps2=ps2):
                        st2 = st2pool.tile([M, TCH], f32, tag="q2st")
                        nc.scalar.add(st2[:], ps2[:], -C_BIAS)
                        nc.sync.dma_start(
                            q2[b * M:(b + 1) * M, c * TCH:(c + 1) * TCH],
                            st2[:])
                    closures.append(tail)
                return closures

            def p2_build(c):
                t0 = c * TCH
                if c == 0:
                    nc.vector.tensor_scalar(
                        p2[:, 0:1], q2[:, 0:1], C_BIAS, None, AOp.add)
                else:
                    nc.vector.scalar_tensor_tensor(
                        p2[:, t0:t0 + 1], q2[:, t0 - 1:t0], -ALPHA,
                        q2[:, t0:t0 + 1], AOp.mult, AOp.add)
                nc.vector.scalar_tensor_tensor(
                    p2[:, t0 + 1:t0 + TCH], q2[:, t0:t0 + TCH - 1], -ALPHA,
                    q2[:, t0 + 1:t0 + TCH], AOp.mult, AOp.add)

            # scan segment c = steps [c*TCH-1 (or 0), (c+1)*TCH-1) producing
            # V cols [c*TCH, (c+1)*TCH); reads p cols within chunks <= c.
            seg = []
            start = 0
            for c in range(NTC):
                end = (c + 1) * TCH - 1 if c < NTC - 1 else T - 1
                seg.append((start, end))
                start = end

            # ---------------- one full execution ---------------------------
            def emit_once():
                for i in range(KC):
                    nc.sync.dma_start(
                        w1t[:, i * N:(i + 1) * N],
                        w1_d[i * KCH:(i + 1) * KCH, :])
                nc.sync.dma_start(w2t[:], w2_d[:])
                # prologue
                mm1_chunk_pe(0, defer_bias=False)
                p1_build(0)
                nc.vector.memset(v1_3[:, :, 0], 0.0)
                nc.vector.tensor_copy(za[:], p1_3[:, :, 0])

                # phase c: PE runs mm2 chunk c-1 then mm1 chunk c+1 while
                # DVE/Act run scan1 seg c interleaved with scan2 seg c-1.
                # Slow-engine tails (st2 bias+DMA, qb bias) are sprinkled
                # into the scan stream so no engine queue stalls behind an
                # unready op.
                PREFIX = 32  # scan1-only steps before p2-build(c-1)
                for c in range(NTC + 1):
                    st2_extras = list(mm2_mm(c - 1)) if 1 <= c <= NTC else []
                    qb_extras = (mm1_chunk_pe(c + 1)
                                 if c + 1 <= NTC - 1 else [])
                    s1 = range(*seg[c]) if c < NTC else range(0)
                    s2 = range(*seg[c - 1]) if c >= 1 else range(0)
                    it1, it2 = iter(s1), iter(s2)
                    if c >= 1:
                        # scan1-only prefix; st2 tails every 3rd step
                        for k in range(PREFIX):
                            t1 = next(it1, None)
                            if t1 is not None:
                                scan1_step(t1)
                            if k % 3 == 2 and st2_extras:
                                st2_extras.pop(0)()
                        while st2_extras:
                            st2_extras.pop(0)()
                        p2_build(c - 1)
                        if c == 1:
                            nc.vector.memset(v2[:, 0:1], 0.0)
                            nc.vector.tensor_copy(za2[:], p2[:, 0:1])
                    k = 0
                    while True:
                        t1 = next(it1, None)
                        if t1 is not None:
                            scan1_step(t1)
                        t2_ = next(it2, None)
                        if t2_ is not None:
                            scan2_step(t2_)
                        if k % 16 == 15 and qb_extras:
                            qb_extras.pop(0)()
                        k += 1
                        if t1 is None and t2_ is None:
                            break
                    while qb_extras:
                        qb_extras.pop(0)()
                    if c + 1 <= NTC - 1:
                        p1_build(c + 1)

                # unscale into q2 (dead) and DMA out
                nc.vector.tensor_scalar(q2[:, 0:T], v2[:], 1.0 / K_SC, None,
                                        AOp.mult)
                for b in range(BL):
                    nc.sync.dma_start(
                        out_d[b].rearrange("t m -> m t"),
                        q2[b * M:(b + 1) * M, 0:T])

            for _ in range(reps):
                emit_once()

    return nc


def _build_kernel_v4(scan_steps=None, reps=1):
    """Merged-scan variant: scan2's 80 state elements live on partitions
    100-109 (10 neurons x 8 sample-slots), time-shifted +512 columns, so ONE
    set of 5 all-DVE ops per combined step advances BOTH scans ([110, 8]
    operands, no cross-engine hop in the recurrence chain):

        u  = V*V                     (tensor_tensor)
        t2 = alpha*Z - V             (scalar_tensor_tensor)
        Z' = t2 + p[col t+1]         (tensor_tensor)
        m  = (u - A)*V               (scalar_tensor_tensor)
        V' = beta*Z - m              (scalar_tensor_tensor)

    Layouts (sample/slot-major): rows 0-99 of pp/vv hold p1/V1 at col
    b*TP|b*T + t; rows 100-109 hold p2/V2 for slot b at col b*TP|b*T +
    tau + 512 (tau = scan2 time = t - 512). Slot b's tail columns overlap
    slot b+1's head-garbage region; garbage is written before real data
    arrives and never read for output. A 512-step tail (rows 100-109 only)
    finishes scan2 after scan1 ends. mm1/mm2/p-builds keep the v3 chunk
    pipeline; PE/Act/DMA tails are sprinkled into the scan stream."""
    assert scan_steps is None
    import concourse.bass as bass
    import concourse.tile as tile
    from concourse import mybir

    f32 = mybir.dt.float32
    AOp = mybir.AluOpType

    nc = bass.Bass()
    _cb = nc.alloc_sbuf_tensor("const-cbias", [128, 1], f32)
    nc.gpsimd.memset(_cb.ap(), -C_BIAS)
    nc.const_aps.aps[(f32, -C_BIAS)] = _cb.ap()
    nc.all_engine_barrier()

    bt_d = nc.declare_dram_parameter("batchT", [BL, D, T], f32, isOutput=False)
    w1_d = nc.declare_dram_parameter("W1T", [D, N], f32, isOutput=False)
    w2_d = nc.declare_dram_parameter("W2T", [N, M], f32, isOutput=False)
    out_d = nc.declare_dram_parameter("out", [BL, T, M], f32, isOutput=True)

    NR = N + M          # 110 combined rows
    SH = TCH            # scan2 time shift (one chunk)

    with tile.TileContext(nc) as tc:
        with (
            tc.tile_pool(name="const", bufs=1) as cpool,
            tc.tile_pool(name="bt", bufs=4) as btpool,
            tc.tile_pool(name="qs", bufs=8) as qspool,
            tc.tile_pool(name="ps1", bufs=4, space="PSUM") as ps1pool,
            tc.tile_pool(name="ps2", bufs=4, space="PSUM") as ps2pool,
            tc.tile_pool(name="big", bufs=1) as bigpool,
            tc.tile_pool(name="small", bufs=6) as spool,
            tc.tile_pool(name="st2p", bufs=8) as st2pool,
            tc.tile_pool(name="p2cp", bufs=3) as p2cpool,
            tc.tile_pool(name="state", bufs=2) as stpool,
        ):
            w1t = cpool.tile([KCH, KC * N], f32)
            w2t = cpool.tile([N, M], f32)
            lastq = cpool.tile([N, BL], f32)

            # one extra slot of width so tail columns (slot b spilling into
            # slot b+1's range, and slot 7 into the pad) stay in-bounds
            pp = bigpool.tile([NR, (BL + 1) * TP], f32)   # p1 | p2 (shifted)
            vv = bigpool.tile([NR, (BL + 1) * T], f32)    # V1 | V2 (shifted)
            vout = bigpool.tile([NR, T], f32)             # unscaled V2 staging

            za = stpool.tile([NR, BL], f32, tag="za")
            zb = stpool.tile([NR, BL], f32, tag="zb")

            # ---------------- emission helpers -----------------------------
            qbs = {}

            def mm1_chunk_pe(c, defer_bias=True):
                closures = []
                for b in range(BL):
                    ps = ps1pool.tile([N, TCH], f32)
                    for i in range(KC):
                        bt = btpool.tile([KCH, TCH], f32)
                        nc.sync.dma_start(
                            bt[:],
                            bt_d[b, i * KCH:(i + 1) * KCH,
                                 c * TCH:(c + 1) * TCH],
                        )
                        nc.tensor.matmul(
                            ps[:], lhsT=w1t[:, i * N:(i + 1) * N], rhs=bt[:],
                            start=(i == 0), stop=(i == KC - 1),
                        )

                    def bias(b=b, ps=ps):
                        qb = qspool.tile([N, TCH], f32, tag="qhat")
                        nc.scalar.add(qb[:], ps[:], -C_BIAS)
                        qbs[(b, c)] = qb
                    if defer_bias:
                        closures.append(bias)
                    else:
                        bias()
                return closures

            def p1_build(c):
                for b in range(BL):
                    qb = qbs.pop((b, c))
                    s = pp[0:N, b * TP:(b + 1) * TP]
                    t0 = c * TCH
                    if c == 0:
                        nc.vector.tensor_scalar(
                            s[:, 0:1], qb[:, 0:1], C_BIAS, None, AOp.add)
                    else:
                        nc.vector.scalar_tensor_tensor(
                            s[:, t0:t0 + 1], lastq[:, b:b + 1], -ALPHA,
                            qb[:, 0:1], AOp.mult, AOp.add)
                    nc.vector.scalar_tensor_tensor(
                        s[:, t0 + 1:t0 + TCH], qb[:, 0:TCH - 1], -ALPHA,
                        qb[:, 1:TCH], AOp.mult, AOp.add)
                    if c < NTC - 1:
                        nc.vector.tensor_copy(
                            lastq[:, b:b + 1], qb[:, TCH - 1:TCH])

            st2s = {}
            lastq2 = cpool.tile([M, BL], f32)

            def mm2_mm(c):
                """PE mm2 chunk c -> PSUM rows 0-9; Act bias-adds returned
                as closures (st2 on rows 0-9, start-partition 0)."""
                closures = []
                for b in range(BL):
                    ps2 = ps2pool.tile([M, TCH], f32)
                    nc.tensor.matmul(
                        ps2[:], lhsT=w2t[:],
                        rhs=vv[0:N, b * T + c * TCH: b * T + (c + 1) * TCH],
                        start=True, stop=True)

                    def tail(b=b, ps2=ps2):
                        st2 = st2pool.tile([M, TCH], f32, tag="q2st")
                        nc.scalar.add(st2[:], ps2[:], -C_BIAS)
                        st2s[(b, c)] = st2
                    closures.append(tail)
                return closures

            def p2_build(c):
                """Build p2 chunk c on rows 0-9, then DMA it to pp rows
                100-109 at cols b*TP + SH + tau (DVE/Act cannot address a
                start partition of 100; DMA can)."""
                t0 = c * TCH
                for b in range(BL):
                    st2 = st2s.pop((b, c))
                    q = st2[:]
                    p2c = p2cpool.tile([M, TCH], f32, tag="p2c")
                    if c == 0:
                        nc.vector.tensor_scalar(
                            p2c[:, 0:1], q[:, 0:1], C_BIAS, None, AOp.add)
                    else:
                        nc.vector.scalar_tensor_tensor(
                            p2c[:, 0:1], lastq2[:, b:b + 1], -ALPHA,
                            q[:, 0:1], AOp.mult, AOp.add)
                    nc.vector.scalar_tensor_tensor(
                        p2c[:, 1:TCH], q[:, 0:TCH - 1], -ALPHA, q[:, 1:TCH],
                        AOp.mult, AOp.add)
                    if c < NTC - 1:
                        nc.vector.tensor_copy(
                            lastq2[:, b:b + 1], q[:, TCH - 1:TCH])
                    nc.sync.dma_start(
                        pp[N:NR, b * TP + SH + t0:b * TP + SH + t0 + TCH],
                        p2c[:])

            pp3 = pp[:].rearrange("p (b t) -> p b t", b=BL + 1)
            vv3 = vv[:].rearrange("p (b t) -> p b t", b=BL + 1)

            def vcol(rows, t):
                if t < T:
                    return vv3[rows, 0:BL, t]
                return vv3[rows, 1:BL + 1, t - T]

            def pcol(rows, t):
                if t < TP:
                    return pp3[rows, 0:BL, t]
                return pp3[rows, 1:BL + 1, t - TP]

            def step(t, rows=slice(0, NR)):
                """One combined scan step (all-DVE, 5 ops)."""
                zs, zn = (za, zb) if t % 2 == 0 else (zb, za)
                vt = vcol(rows, t)
                vn = vcol(rows, t + 1)
                pc = pcol(rows, t + 1)
                zs = zs[rows, :]
                zn = zn[rows, :]
                u_t = spool.tile([NR, BL], f32, tag="u")
                u = u_t[rows, :]
                nc.vector.tensor_tensor(u, vt, vt, AOp.mult)
                t2_t = spool.tile([NR, BL], f32, tag="t2")
                t2 = t2_t[rows, :]
                nc.vector.scalar_tensor_tensor(
                    t2, zs, ALPHA, vt, AOp.mult, AOp.subtract)
                nc.vector.tensor_tensor(zn, t2, pc, AOp.add)
                m_t = spool.tile([NR, BL], f32, tag="m")
                m = m_t[rows, :]
                nc.vector.scalar_tensor_tensor(
                    m, u, A_CONST, vt, AOp.subtract, AOp.mult)
                nc.vector.scalar_tensor_tensor(
                    vn, zs, BETA, m, AOp.mult, AOp.subtract)

            P0 = 96   # lowest legal start partition covering rows 100-109
            sav1 = cpool.tile([NR, BL], f32)
            sav2 = cpool.tile([NR, BL], f32)

            def bootstrap2():
                """Reset rows 100-109 state for scan2 start (combined t=SH).
                Runs between step(SH-1) and step(SH); za holds Z(SH). Engine
                APs must start at partition 96, so the live scan1 rows 96-99
                are saved and restored around the wide writes."""
                nc.vector.tensor_copy(sav1[P0:N, :], vv3[P0:N, 0:BL, SH])
                nc.vector.memset(vv3[P0:NR, 0:BL, SH], 0.0)
                nc.vector.tensor_copy(vv3[P0:N, 0:BL, SH], sav1[P0:N, :])
                nc.vector.tensor_copy(sav2[P0:N, :], za[P0:N, :])
                nc.vector.tensor_copy(za[P0:NR, :], pp3[P0:NR, 0:BL, SH])
                nc.vector.tensor_copy(za[P0:N, :], sav2[P0:N, :])

            # seg[c]: combined steps of phase c (scan1 rows produce V1 cols
            # [c*TCH, (c+1)*TCH); scan2 rows trail by SH columns)
            seg = []
            start = 0
            for c in range(NTC):
                end = (c + 1) * TCH - 1 if c < NTC - 1 else T - 1
                seg.append((start, end))
                start = end

            PREF2 = 64   # steps into a phase before p2_build(c-1)

            def emit_once():
                for i in range(KC):
                    nc.sync.dma_start(
                        w1t[:, i * N:(i + 1) * N],
                        w1_d[i * KCH:(i + 1) * KCH, :])
                nc.sync.dma_start(w2t[:], w2_d[:])
                # head-garbage regions read by rows 100-109 before bootstrap
                # (start partition 96; rows 96-99 are rebuilt afterwards)
                nc.gpsimd.memset(pp[P0:NR, :], 0.0)
                nc.gpsimd.memset(vv[P0:NR, :], 0.0)
                mm1_chunk_pe(0, defer_bias=False)
                p1_build(0)
                nc.vector.memset(vv3[0:N, 0:BL, 0], 0.0)
                nc.vector.tensor_copy(za[:], pp3[:, 0:BL, 0])

                for c in range(NTC):
                    st2_extras = list(mm2_mm(c - 1)) if c >= 1 else []
                    qb_extras = (mm1_chunk_pe(c + 1)
                                 if c + 1 <= NTC - 1 else [])
                    k = 0
                    for t in range(*seg[c]):
                        if t == SH:
                            bootstrap2()
                        step(t)
                        if k % 3 == 2 and st2_extras:
                            st2_extras.pop(0)()
                        if k == PREF2 and c >= 1:
                            while st2_extras:
                                st2_extras.pop(0)()
                            p2_build(c - 1)
                        if k % 16 == 15 and qb_extras:
                            qb_extras.pop(0)()
                        k += 1
                    while st2_extras:
                        st2_extras.pop(0)()
                    while qb_extras:
                        qb_extras.pop(0)()
                    if c + 1 <= NTC - 1:
                        p1_build(c + 1)

                # tail: scan2 rows only, combined t in [T-1, T-1+SH)
                st2_extras = list(mm2_mm(NTC - 1))
                k = 0
                for t in range(T - 1, T - 1 + SH):
                    step(t, rows=slice(P0, NR))
                    if k % 3 == 2 and st2_extras:
                        st2_extras.pop(0)()
                    if k == PREF2:
                        while st2_extras:
                            st2_extras.pop(0)()
                        p2_build(NTC - 1)
                    k += 1

                # unscale + transposed DMA out, per sample slot
                for b in range(BL):
                    nc.vector.tensor_scalar(
                        vout[P0:NR, :], vv[P0:NR, b * T + SH: b * T + SH + T],
                        1.0 / K_SC, None, AOp.mult)
                    nc.sync.dma_start(
                        out_d[b].rearrange("t m -> m t"), vout[N:NR, :])

            for _ in range(reps):
                emit_once()

    return nc


def _build_kernel(scan_steps=None):
    import concourse.bass as bass
    import concourse.tile as tile
    from concourse import mybir

    f32 = mybir.dt.float32
    AOp = mybir.AluOpType

    nc = bass.Bass()
    # register the bias constant for ScalarE add (activation bias const-AP)
    _cb = nc.alloc_sbuf_tensor("const-cbias", [128, 1], f32)
    nc.gpsimd.memset(_cb.ap(), -C_BIAS)
    nc.const_aps.aps[(f32, -C_BIAS)] = _cb.ap()
    nc.all_engine_barrier()

    bt_d = nc.declare_dram_parameter("batchT", [BL, D, T], f32, isOutput=False)
    w1_d = nc.declare_dram_parameter("W1T", [D, N], f32, isOutput=False)
    w2_d = nc.declare_dram_parameter("W2T", [N, M], f32, isOutput=False)
    out_d = nc.declare_dram_parameter("out", [BL, T, M], f32, isOutput=True)

    with tile.TileContext(nc) as tc:
        with (
            tc.tile_pool(name="const", bufs=1) as cpool,
            tc.tile_pool(name="bt", bufs=3) as btpool,
            tc.tile_pool(name="qs", bufs=3) as qspool,
            tc.tile_pool(name="ps1", bufs=4, space="PSUM") as ps1pool,
            tc.tile_pool(name="ps2", bufs=2, space="PSUM") as ps2pool,
            tc.tile_pool(name="big", bufs=1) as bigpool,
            tc.tile_pool(name="small", bufs=3) as spool,
            tc.tile_pool(name="state", bufs=2) as stpool,
        ):
            w1t = cpool.tile([KCH, KC * N], f32)
            for i in range(KC):
                nc.sync.dma_start(
                    w1t[:, i * N:(i + 1) * N], w1_d[i * KCH:(i + 1) * KCH, :]
                )
            w2t = cpool.tile([N, M], f32)
            nc.sync.dma_start(w2t[:], w2_d[:])

            p1 = bigpool.tile([N, BL * TP], f32)    # scan-1 p-stream, per sample
            v1 = bigpool.tile([N, BL * T], f32)     # scan-1 output (Vt1)
            q2 = bigpool.tile([BL * M, T], f32)     # q-hat-2, reused as Vt2
            p2 = bigpool.tile([BL * M, TP], f32)    # scan-2 p-stream

            p1_3 = p1[:].rearrange("p (b t) -> p b t", b=BL)
            v1_3 = v1[:].rearrange("p (b t) -> p b t", b=BL)

            # ---------------- mm1 + per-sample stream build ------------------
            for b in range(BL):
                qb = qspool.tile([N, T], f32, tag="qhat")
                for c in range(NTC):
                    ps = ps1pool.tile([N, TCH], f32)
                    for i in range(KC):
                        bt = btpool.tile([KCH, TCH], f32)
                        nc.sync.dma_start(
                            bt[:],
                            bt_d[b, i * KCH:(i + 1) * KCH,
                                 c * TCH:(c + 1) * TCH],
                        )
                        nc.tensor.matmul(
                            ps[:], lhsT=w1t[:, i * N:(i + 1) * N], rhs=bt[:],
                            start=(i == 0), stop=(i == KC - 1),
                        )
                    nc.scalar.add(qb[:, c * TCH:(c + 1) * TCH], ps[:], -C_BIAS)
                s = p1[:, b * TP:(b + 1) * TP]
                nc.vector.tensor_scalar(
                    s[:, 0:1], qb[:, 0:1], C_BIAS, None, AOp.add)
                nc.vector.scalar_tensor_tensor(
                    s[:, 1:T], qb[:, 0:T - 1], -ALPHA, qb[:, 1:T],
                    AOp.mult, AOp.add)
                nc.gpsimd.memset(s[:, T:T + 1], 0.0)

            # ---------------- scan 1 (all samples per instruction) ----------
            # state cols: V_t lives in v1[:, :, t]; Z in ping-pong tiles.
            nc.vector.memset(v1_3[:, :, 0], 0.0)
            za = stpool.tile([N, BL], f32, tag="za")
            zb = stpool.tile([N, BL], f32, tag="zb")
            # Z_0 = qhat_0 + c  (= p-stream col 0 + ... col0 holds qhat0 + c)
            nc.vector.tensor_copy(za[:], p1_3[:, :, 0])
            SQ = mybir.ActivationFunctionType.Square
            _S1 = (T - 1) if scan_steps is None else scan_steps
            for t in range(_S1):
                zs, zn = (za, zb) if t % 2 == 0 else (zb, za)
                vt = v1_3[:, :, t]
                u = spool.tile([N, BL], f32, tag="u")
                nc.scalar.activation(u[:], vt, SQ)        # u = V^2 (ScalarE)
                m = spool.tile([N, BL], f32, tag="m")
                # m' = (u - A) * V  == -(A - u)*V
                nc.vector.scalar_tensor_tensor(
                    m[:], u[:], A_CONST, vt, AOp.subtract, AOp.mult)
                # V' = beta*Z - m'
                nc.vector.scalar_tensor_tensor(
                    v1_3[:, :, t + 1], zs[:], BETA, m[:], AOp.mult,
                    AOp.subtract)
                t2 = spool.tile([N, BL], f32, tag="t2")
                nc.vector.scalar_tensor_tensor(
                    t2[:], zs[:], ALPHA, vt, AOp.mult, AOp.subtract)
                nc.vector.tensor_tensor(zn[:], t2[:], p1_3[:, :, t + 1],
                                        AOp.add)

            # ---------------- mm2 ------------------------------------------
            for b in range(BL):
                for c in range(NTC):
                    ps2 = ps2pool.tile([M, TCH], f32)
                    nc.tensor.matmul(
                        ps2[:], lhsT=w2t[:],
                        rhs=v1[:, b * T + c * TCH: b * T + (c + 1) * TCH],
                        start=True, stop=True)
                    st2 = spool.tile([M, TCH], f32, tag="q2st")
                    nc.scalar.add(st2[:], ps2[:], -C_BIAS)
                    nc.sync.dma_start(
                        q2[b * M:(b + 1) * M, c * TCH:(c + 1) * TCH], st2[:])

            # ---------------- scan 2 ([80, 1] slices) -----------------------
            nc.vector.tensor_scalar(
                p2[:, 0:1], q2[:, 0:1], C_BIAS, None, AOp.add)
            nc.vector.scalar_tensor_tensor(
                p2[:, 1:T], q2[:, 0:T - 1], -ALPHA, q2[:, 1:T],
                AOp.mult, AOp.add)
            nc.gpsimd.memset(p2[:, T:T + 1], 0.0)

            P2 = BL * M
            za2 = stpool.tile([P2, 1], f32, tag="za2")
            zb2 = stpool.tile([P2, 1], f32, tag="zb2")
            nc.vector.tensor_copy(za2[:], p2[:, 0:1])
            nc.vector.memset(q2[:, 0:1], 0.0)   # V2 col 0 (q2 reused as V2)
            _S2 = (T - 1) if scan_steps is None else scan_steps
            for t in range(_S2):
                zs, zn = (za2, zb2) if t % 2 == 0 else (zb2, za2)
                vt = q2[:, t:t + 1]
                u = spool.tile([P2, 1], f32, tag="u2")
                nc.vector.tensor_tensor(u[:], vt, vt, AOp.mult)
                r = spool.tile([P2, 1], f32, tag="r2")
                nc.vector.tensor_scalar(r[:], u[:], -1.0, A_CONST,
                                        AOp.mult, AOp.add)
                m = spool.tile([P2, 1], f32, tag="m2")
                nc.vector.tensor_tensor(m[:], vt, r[:], AOp.mult)
                nc.vector.scalar_tensor_tensor(
                    q2[:, t + 1:t + 2], zs[:], BETA, m[:], AOp.mult, AOp.add)
                t2 = spool.tile([P2, 1], f32, tag="t22")
                nc.vector.scalar_tensor_tensor(
                    t2[:], zs[:], ALPHA, vt, AOp.mult, AOp.subtract)
                nc.vector.tensor_tensor(zn[:], t2[:], p2[:, t + 1:t + 2],
                                        AOp.add)

            # unscale into p2 (dead) and DMA out
            nc.vector.tensor_scalar(p2[:, 0:T], q2[:], 1.0 / K_SC, None,
                                    AOp.mult)
            for b in range(BL):
                nc.sync.dma_start(
                    out_d[b].rearrange("t m -> m t"),
                    p2[b * M:(b + 1) * M, 0:T])

    return nc


def kernel(batch, W1, W2):
    _install_bir_patch()
    from concourse.bass_utils import run_bass_kernel_spmd

    if "nc" not in _CACHE:
        _CACHE["nc"] = _build_kernel_v3()
    nc = _CACHE["nc"]

    batch = np.asarray(batch, dtype=np.float32)
    W1 = np.asarray(W1, dtype=np.float32)
    W2 = np.asarray(W2, dtype=np.float32)

    w1t = np.ascontiguousarray((KOB * DT * W1).T.astype(np.float32))
    w2t = np.ascontiguousarray(((DT * 0.5 / BETA) * W2).T.astype(np.float32))

    in_maps = []
    for cidx in range(NCORES):
        sl = batch[cidx * BL:(cidx + 1) * BL]
        btT = np.ascontiguousarray(sl.transpose(0, 2, 1))
        in_maps.append({"batchT": btT, "W1T": w1t, "W2T": w2t})

    res = run_bass_kernel_spmd(nc, in_maps, list(range(NCORES)))
    out = np.concatenate([res.results[i]["out"] for i in range(NCORES)], axis=0)
    return out.astype(np.float32)



# revision 28
# speedup vs baseline: 1.0007x; 1.0007x over previous
"""Trainium2 Bass kernel for nn_FHNet (batch[64,2048,784] @ W1.T -> FHN scan
-> *0.5 @ W2.T -> FHN scan), data-parallel over batch across 8 NeuronCores.

Per core (8 samples):
- mm1 on PE: K=784 in 7 chunks, batch pre-transposed host-side to
  [8, 784, 2048] so the contraction dim lands on partitions. Weights
  pre-scaled host-side so all constant folds (dt, 0.5, k/beta rescale,
  gamma shift) are free.
- FHN scans as per-step stock DVE ops on [100, 8] (scan1: feature on
  partitions, samples on free) / [80, 1] (scan2) column slices.
  Rescaled recurrence (Vt = k*V, Z = (k/beta)*(q - W)):
      Vt' = Vt*(A - Vt^2) + beta*Z ;  Z' = alpha*Z - Vt + p_t
  with p precomputed in bulk from the matmul outputs.
- mm2 on PE (K=100, single matmul per 512-token chunk).
- Output via strided DMA ([10, T] SBUF -> [T, 10] DRAM).

This container's walrus accepts at most ONE sync wait per engine
instruction; Tile emits more. `_split_multi_waits` hoists extras into
preceding same-engine EventSemaphore instructions (in-order execution
keeps semantics identical).
"""
import json
import sys
import numpy as np

sys.path.insert(0, "/opt/trn_rl_repo")

# ---------------- constants ----------------
DT = 0.04
A_CONST = float(1.0 + DT)
ALPHA = float(1.0 - DT * 0.08 * 0.8)
BETA = float(DT * DT * 0.08)
GAMMA = float(DT * DT * 0.08 * 0.7)
K_SC = float(np.sqrt(DT / 3.0))
KOB = float(K_SC / BETA)
C_BIAS = float(KOB * GAMMA / (1.0 - ALPHA))

B, T, D, N, M = 64, 2048, 784, 100, 10
NCORES = 8
BL = B // NCORES
KC = 7
KCH = D // KC          # 112
TCH = 512
NTC = T // TCH
TP = T + 1

_CACHE = {}


# ------------- walrus single-wait workaround -------------
def _split_multi_waits(bir_json_bytes: bytes) -> bytes:
    d = json.loads(bir_json_bytes)
    for fn in d.get("functions", []):
        for blk in fn.get("blocks", []):
            out = []
            for inst in blk.get("instructions", []):
                si = inst.get("sync_info")
                waits = (si or {}).get("on_wait") or []
                if len(waits) > 1:
                    for k, w in enumerate(waits[:-1]):
                        ev = {
                            "engine": inst["engine"],
                            "ins": [],
                            "outs": [],
                            "name": f"{inst['name']}_hw{k}",
                            "opcode": "EventSemaphore",
                            "sync_info": {"on_update": [], "on_wait": [w]},
                        }
                        if "debug" in inst:
                            ev["debug"] = inst["debug"]
                        out.append(ev)
                    si["on_wait"] = waits[-1:]
                out.append(inst)
            blk["instructions"] = out
    return json.dumps(d).encode()


def _install_bir_patch():
    import concourse.bass_utils as bu
    import concourse.bass2jax as b2j

    if getattr(bu, "_multiwait_patched", False):
        return
    orig = bu.compile_bir_kernel

    def patched(bir_json, tmpdir, neff_name="file.neff"):
        if isinstance(bir_json, str):
            bir_json = bir_json.encode()
        return orig(_split_multi_waits(bir_json), tmpdir, neff_name=neff_name)

    bu.compile_bir_kernel = patched
    bu._multiwait_patched = True
    b2j.compile_bir_kernel = patched


def _register_fhn_ops():
    """Register the two fused FHN-step custom DVE ops (documented extension
    point: dve_ops.OPS + _SUB_OPCODE_FOR_NAME + CUSTOM_DVE_SPECS).

    Reformulated recurrence (scaled vars Vt, H; G := Vt*(A - Vt^2)):
        Vt[t+2] = G(Vt[t+1]) + alpha*Vt[t+1] - H[t]          (STEP_V)
        H[t+1]  = alpha*G(Vt[t+1]) + beta*(Vt[t+1] - p[t+1]) (STEP_H)
    equivalent to the baseline (Vt, Z) system with H[t] =
    alpha*G(Vt[t]) + beta*Vt[t] - beta*p[t]; bootstrap:
        Vt[1] = beta*p1col0,  H[0] = -beta*p1col1.
    """
    import concourse.dve_ops as dops
    from concourse.dve_spec import Spec, Src0, Src1, C0, C1, C2, sq, lower, _has_src1
    from concourse.dve_uop import DveOpSpec

    if "FHN_STEP_V_ANT" in dops._SUB_OPCODE_FOR_NAME:
        return

    defs = [
        ("FHN_STEP_V_ANT",
         (C0 - sq(Src0)) * Src0 + C1 * Src0 - Src1,
         lambda in0, in1, s0, s1, imm2:
             (np.float32(s0) - in0 * in0) * in0 + np.float32(s1) * in0 - in1),
        ("FHN_STEP_H_ANT",
         C0 * ((C2 - sq(Src0)) * Src0) + C1 * (Src0 - Src1),
         lambda in0, in1, s0, s1, imm2:
             np.float32(s0) * ((np.float32(imm2) - in0 * in0) * in0)
             + np.float32(s1) * (in0 - in1)),
    ]
    for name, body, ref in defs:
        row = max(dops._SUB_OPCODE_FOR_NAME.values()) + 1
        assert row < 0x20
        spec = Spec(body=body, reference=ref)
        shas = {}
        for ver in ("v3", "v4"):
            uops = lower(spec, ver=ver)
            shas[ver] = DveOpSpec(
                name=name, opcode=row, uops=uops, rd1_en=_has_src1(spec)
            ).sha(ver)
        op = dops.DveOp(name, spec, subdim=False, uops_sha=shas)
        dops._SUB_OPCODE_FOR_NAME[name] = row
        dops.OPS.append(op)
        dops.CUSTOM_DVE_SPECS[name] = spec


def _build_kernel_v2(scan_steps=None):
    """Same structure as v1 but the FHN scans run 2 fused custom-DVE
    instructions per time step (vs 5 ops incl. a ScalarE round-trip)."""
    _register_fhn_ops()
    import concourse.bass as bass
    import concourse.tile as tile
    from concourse import mybir
    from concourse.dve_ops import CUSTOM_DVE_SPECS, OPS

    step_v = next(o for o in OPS if o.name == "FHN_STEP_V_ANT")
    step_h = next(o for o in OPS if o.name == "FHN_STEP_H_ANT")

    f32 = mybir.dt.float32
    AOp = mybir.AluOpType

    nc = bass.Bass()
    _cb = nc.alloc_sbuf_tensor("const-cbias", [128, 1], f32)
    nc.gpsimd.memset(_cb.ap(), -C_BIAS)
    nc.const_aps.aps[(f32, -C_BIAS)] = _cb.ap()
    nc.all_engine_barrier()

    bt_d = nc.declare_dram_parameter("batchT", [BL, D, T], f32, isOutput=False)
    w1_d = nc.declare_dram_parameter("W1T", [D, N], f32, isOutput=False)
    w2_d = nc.declare_dram_parameter("W2T", [N, M], f32, isOutput=False)
    out_d = nc.declare_dram_parameter("out", [BL, T, M], f32, isOutput=True)

    with tile.TileContext(nc) as tc:
        with (
            tc.tile_pool(name="const", bufs=1) as cpool,
            tc.tile_pool(name="bt", bufs=3) as btpool,
            tc.tile_pool(name="qs", bufs=3) as qspool,
            tc.tile_pool(name="ps1", bufs=4, space="PSUM") as ps1pool,
            tc.tile_pool(name="ps2", bufs=2, space="PSUM") as ps2pool,
            tc.tile_pool(name="big", bufs=1) as bigpool,
            tc.tile_pool(name="small", bufs=3) as spool,
            tc.tile_pool(name="state", bufs=2) as stpool,
        ):
            w1t = cpool.tile([KCH, KC * N], f32)
            for i in range(KC):
                nc.sync.dma_start(
                    w1t[:, i * N:(i + 1) * N], w1_d[i * KCH:(i + 1) * KCH, :]
                )
            w2t = cpool.tile([N, M], f32)
            nc.sync.dma_start(w2t[:], w2_d[:])

            p1 = bigpool.tile([N, BL * TP], f32)
            v1 = bigpool.tile([N, BL * T], f32)
            q2 = bigpool.tile([BL * M, T], f32)
            p2 = bigpool.tile([BL * M, TP], f32)

            p1_3 = p1[:].rearrange("p (b t) -> p b t", b=BL)
            v1_3 = v1[:].rearrange("p (b t) -> p b t", b=BL)

            # ---------------- mm1 + per-sample p-stream build ---------------
            for b in range(BL):
                qb = qspool.tile([N, T], f32, tag="qhat")
                for c in range(NTC):
                    ps = ps1pool.tile([N, TCH], f32)
                    for i in range(KC):
                        bt = btpool.tile([KCH, TCH], f32)
                        nc.sync.dma_start(
                            bt[:],
                            bt_d[b, i * KCH:(i + 1) * KCH,
                                 c * TCH:(c + 1) * TCH],
                        )
                        nc.tensor.matmul(
                            ps[:], lhsT=w1t[:, i * N:(i + 1) * N], rhs=bt[:],
                            start=(i == 0), stop=(i == KC - 1),
                        )
                    nc.scalar.add(qb[:, c * TCH:(c + 1) * TCH], ps[:], -C_BIAS)
                s = p1[:, b * TP:(b + 1) * TP]
                nc.vector.tensor_scalar(
                    s[:, 0:1], qb[:, 0:1], C_BIAS, None, AOp.add)
                nc.vector.scalar_tensor_tensor(
                    s[:, 1:T], qb[:, 0:T - 1], -ALPHA, qb[:, 1:T],
                    AOp.mult, AOp.add)

            # ---------------- scan 1: 2 fused ops per step ------------------
            _S1 = (T - 2) if scan_steps is None else min(scan_steps, T - 2)
            nc.vector.memset(v1_3[:, :, 0], 0.0)
            nc.vector.tensor_scalar(v1_3[:, :, 1], p1_3[:, :, 0], BETA, None,
                                    AOp.mult)
            ha = stpool.tile([N, BL], f32, tag="ha")
            hb = stpool.tile([N, BL], f32, tag="hb")
            nc.vector.tensor_scalar(ha[:], p1_3[:, :, 1], -BETA, None,
                                    AOp.mult)
            for t in range(_S1):
                hs, hn = (ha, hb) if t % 2 == 0 else (hb, ha)
                nc.vector._custom_dve(
                    step_v, out=v1_3[:, :, t + 2], in0=v1_3[:, :, t + 1],
                    in1=hs[:], s0=A_CONST, s1=ALPHA)
                if t < _S1 - 1:
                    nc.vector._custom_dve(
                        step_h, out=hn[:], in0=v1_3[:, :, t + 1],
                        in1=p1_3[:, :, t + 2], s0=ALPHA, s1=BETA, imm2=A_CONST)

            # ---------------- mm2 ------------------------------------------
            for b in range(BL):
                for c in range(NTC):
                    ps2 = ps2pool.tile([M, TCH], f32)
                    nc.tensor.matmul(
                        ps2[:], lhsT=w2t[:],
                        rhs=v1[:, b * T + c * TCH: b * T + (c + 1) * TCH],
                        start=True, stop=True)
                    st2 = spool.tile([M, TCH], f32, tag="q2st")
                    nc.scalar.add(st2[:], ps2[:], -C_BIAS)
                    nc.sync.dma_start(
                        q2[b * M:(b + 1) * M, c * TCH:(c + 1) * TCH], st2[:])

            # ---------------- scan 2 ([80, 1] slices) -----------------------
            nc.vector.tensor_scalar(
                p2[:, 0:1], q2[:, 0:1], C_BIAS, None, AOp.add)
            nc.vector.scalar_tensor_tensor(
                p2[:, 1:T], q2[:, 0:T - 1], -ALPHA, q2[:, 1:T],
                AOp.mult, AOp.add)

            P2 = BL * M
            _S2 = (T - 2) if scan_steps is None else min(scan_steps, T - 2)
            nc.vector.memset(q2[:, 0:1], 0.0)
            nc.vector.tensor_scalar(q2[:, 1:2], p2[:, 0:1], BETA, None,
                                    AOp.mult)
            h2a = stpool.tile([P2, 1], f32, tag="h2a")
            h2b = stpool.tile([P2, 1], f32, tag="h2b")
            nc.vector.tensor_scalar(h2a[:], p2[:, 1:2], -BETA, None, AOp.mult)
            for t in range(_S2):
                hs, hn = (h2a, h2b) if t % 2 == 0 else (h2b, h2a)
                nc.vector._custom_dve(
                    step_v, out=q2[:, t + 2:t + 3], in0=q2[:, t + 1:t + 2],
                    in1=hs[:], s0=A_CONST, s1=ALPHA)
                if t < _S2 - 1:
                    nc.vector._custom_dve(
                        step_h, out=hn[:], in0=q2[:, t + 1:t + 2],
                        in1=p2[:, t + 2:t + 3], s0=ALPHA, s1=BETA,
                        imm2=A_CONST)

            # unscale into p2 (dead) and DMA out
            nc.vector.tensor_scalar(p2[:, 0:T], q2[:], 1.0 / K_SC, None,
                                    AOp.mult)
            for b in range(BL):
                nc.sync.dma_start(
                    out_d[b].rearrange("t m -> m t"),
                    p2[b * M:(b + 1) * M, 0:T])

    return nc


def _build_kernel_v3(scan_steps=None, reps=1):
    """Chunk-pipelined: mm1 chunk c+1 (PE/DMA) overlaps scan1 segment c
    (Act square + 4 DVE ops/step); mm2 chunk c (PE) runs between segments;
    scan2 segment c-1 (same 5-op pattern) interleaves instruction-by-
    instruction with scan1 segment c so the two chains fill each other's
    cross-engine latency. Scan2 writes V2 to a separate tile (q2 keeps
    qhat2 for the chunked p2 builds).

    reps > 1 repeats the whole computation (including all DMA) inside one
    NEFF — used by test.py to amortize per-dispatch overhead when timing;
    every rep recomputes the identical result from DRAM inputs."""
    assert scan_steps is None
    import concourse.bass as bass
    import concourse.tile as tile
    from concourse import mybir

    f32 = mybir.dt.float32
    AOp = mybir.AluOpType
    SQ = mybir.ActivationFunctionType.Square

    nc = bass.Bass()
    _cb = nc.alloc_sbuf_tensor("const-cbias", [128, 1], f32)
    nc.gpsimd.memset(_cb.ap(), -C_BIAS)
    nc.const_aps.aps[(f32, -C_BIAS)] = _cb.ap()
    nc.all_engine_barrier()

    bt_d = nc.declare_dram_parameter("batchT", [BL, D, T], f32, isOutput=False)
    w1_d = nc.declare_dram_parameter("W1T", [D, N], f32, isOutput=False)
    w2_d = nc.declare_dram_parameter("W2T", [N, M], f32, isOutput=False)
    out_d = nc.declare_dram_parameter("out", [BL, T, M], f32, isOutput=True)

    P2 = BL * M

    with tile.TileContext(nc) as tc:
        with (
            tc.tile_pool(name="const", bufs=1) as cpool,
            tc.tile_pool(name="bt", bufs=4) as btpool,
            tc.tile_pool(name="qs", bufs=8) as qspool,
            tc.tile_pool(name="ps1", bufs=4, space="PSUM") as ps1pool,
            tc.tile_pool(name="ps2", bufs=4, space="PSUM") as ps2pool,
            tc.tile_pool(name="big", bufs=1) as bigpool,
            tc.tile_pool(name="small", bufs=6) as spool,
            tc.tile_pool(name="st2p", bufs=4) as st2pool,
            tc.tile_pool(name="state", bufs=2) as stpool,
        ):
            w1t = cpool.tile([KCH, KC * N], f32)
            w2t = cpool.tile([N, M], f32)
            lastq = cpool.tile([N, BL], f32)

            p1 = bigpool.tile([N, BL * TP], f32)
            v1 = bigpool.tile([N, BL * T], f32)
            q2 = bigpool.tile([P2, T], f32)     # qhat2 (mm2 output)
            p2 = bigpool.tile([P2, TP], f32)    # scan-2 p-stream
            v2 = bigpool.tile([P2, T], f32)     # scan-2 output (Vt2)

            p1_3 = p1[:].rearrange("p (b t) -> p b t", b=BL)
            v1_3 = v1[:].rearrange("p (b t) -> p b t", b=BL)

            # ---------------- emission helpers -----------------------------
            qbs = {}

            def mm1_chunk_pe(c, defer_bias=True):
                """DMA + PE matmuls for chunk c; Act bias adds returned as
                closures (interleaved into the scan stream so the Act queue
                never stalls the scan squares behind an unready add)."""
                closures = []
                for b in range(BL):
                    ps = ps1pool.tile([N, TCH], f32)
                    for i in range(KC):
                        bt = btpool.tile([KCH, TCH], f32)
                        nc.sync.dma_start(
                            bt[:],
                            bt_d[b, i * KCH:(i + 1) * KCH,
                                 c * TCH:(c + 1) * TCH],
                        )
                        nc.tensor.matmul(
                            ps[:], lhsT=w1t[:, i * N:(i + 1) * N], rhs=bt[:],
                            start=(i == 0), stop=(i == KC - 1),
                        )

                    def bias(b=b, ps=ps):
                        qb = qspool.tile([N, TCH], f32, tag="qhat")
                        nc.scalar.add(qb[:], ps[:], -C_BIAS)
                        qbs[(b, c)] = qb
                    if defer_bias:
                        closures.append(bias)
                    else:
                        bias()
                return closures

            def p1_build(c):
                """DVE p1-stream build for chunk c (deferred past the scans
                emitted earlier in the phase so the DVE never waits on mm1)."""
                for b in range(BL):
                    qb = qbs.pop((b, c))
                    s = p1[:, b * TP:(b + 1) * TP]
                    t0 = c * TCH
                    if c == 0:
                        nc.vector.tensor_scalar(
                            s[:, 0:1], qb[:, 0:1], C_BIAS, None, AOp.add)
                    else:
                        nc.vector.scalar_tensor_tensor(
                            s[:, t0:t0 + 1], lastq[:, b:b + 1], -ALPHA,
                            qb[:, 0:1], AOp.mult, AOp.add)
                    nc.vector.scalar_tensor_tensor(
                        s[:, t0 + 1:t0 + TCH], qb[:, 0:TCH - 1], -ALPHA,
                        qb[:, 1:TCH], AOp.mult, AOp.add)
                    if c < NTC - 1:
                        nc.vector.tensor_copy(
                            lastq[:, b:b + 1], qb[:, TCH - 1:TCH])

            za = stpool.tile([N, BL], f32, tag="za")
            zb = stpool.tile([N, BL], f32, tag="zb")

            def scan1_step(t):
                # Emission order: square first (Act), then the Z-chain ops
                # (independent of the square) so the Act->DVE handoff has
                # two DVE ops of slack before m consumes u.
                zs, zn = (za, zb) if t % 2 == 0 else (zb, za)
                vt = v1_3[:, :, t]
                u = spool.tile([N, BL], f32, tag="u")
                nc.scalar.activation(u[:], vt, SQ)
                t2 = spool.tile([N, BL], f32, tag="t2")
                nc.vector.scalar_tensor_tensor(
                    t2[:], zs[:], ALPHA, vt, AOp.mult, AOp.subtract)
                nc.vector.tensor_tensor(zn[:], t2[:], p1_3[:, :, t + 1],
                                        AOp.add)
                m = spool.tile([N, BL], f32, tag="m")
                nc.vector.scalar_tensor_tensor(
                    m[:], u[:], A_CONST, vt, AOp.subtract, AOp.mult)
                nc.vector.scalar_tensor_tensor(
                    v1_3[:, :, t + 1], zs[:], BETA, m[:], AOp.mult,
                    AOp.subtract)

            za2 = stpool.tile([P2, 1], f32, tag="za2")
            zb2 = stpool.tile([P2, 1], f32, tag="zb2")

            def scan2_step(t):
                zs, zn = (za2, zb2) if t % 2 == 0 else (zb2, za2)
                vt = v2[:, t:t + 1]
                u = spool.tile([P2, 1], f32, tag="u2")
                nc.scalar.activation(u[:], vt, SQ)
                t2 = spool.tile([P2, 1], f32, tag="t22")
                nc.vector.scalar_tensor_tensor(
                    t2[:], zs[:], ALPHA, vt, AOp.mult, AOp.subtract)
                nc.vector.tensor_tensor(zn[:], t2[:], p2[:, t + 1:t + 2],
                                        AOp.add)
                m = spool.tile([P2, 1], f32, tag="m2")
                nc.vector.scalar_tensor_tensor(
                    m[:], u[:], A_CONST, vt, AOp.subtract, AOp.mult)
                nc.vector.scalar_tensor_tensor(
                    v2[:, t + 1:t + 2], zs[:], BETA, m[:], AOp.mult,
                    AOp.subtract)

            def mm2_mm(c):
                """PE matmuls for mm2 chunk c; returns Act/DMA tail closures."""
                closures = []
                for b in range(BL):
                    ps2 = ps2pool.tile([M, TCH], f32)
                    nc.tensor.matmul(
                        ps2[:], lhsT=w2t[:],
                        rhs=v1[:, b * T + c * TCH: b * T + (c + 1) * TCH],
                        start=True, stop=True)

                    def tail(b=b, ps2=ps2):
                        st2 = st2pool.tile([M, TCH], f32, tag="q2st")
                        nc.scalar.add(st2[:], ps2[:], -C_BIAS)
                        nc.sync.dma_start(
                            q2[b * M:(b + 1) * M, c * TCH:(c + 1) * TCH],
                            st2[:])
                    closures.append(tail)
                return closures

            def p2_build(c):
                t0 = c * TCH
                if c == 0:
                    nc.vector.tensor_scalar(
                        p2[:, 0:1], q2[:, 0:1], C_BIAS, None, AOp.add)
                else:
                    nc.vector.scalar_tensor_tensor(
                        p2[:, t0:t0 + 1], q2[:, t0 - 1:t0], -ALPHA,
                        q2[:, t0:t0 + 1], AOp.mult, AOp.add)
                nc.vector.scalar_tensor_tensor(
                    p2[:, t0 + 1:t0 + TCH], q2[:, t0:t0 + TCH - 1], -ALPHA,
                    q2[:, t0 + 1:t0 + TCH], AOp.mult, AOp.add)

            # scan segment c = steps [c*TCH-1 (or 0), (c+1)*TCH-1) producing
            # V cols [c*TCH, (c+1)*TCH); reads p cols within chunks <= c.
            seg = []
            start = 0
            for c in range(NTC):
                end = (c + 1) * TCH - 1 if c < NTC - 1 else T - 1
                seg.append((start, end))
                start = end

            # ---------------- one full execution ---------------------------
            def emit_once():
                for i in range(KC):
                    nc.sync.dma_start(
                        w1t[:, i * N:(i + 1) * N],
                        w1_d[i * KCH:(i + 1) * KCH, :])
                nc.sync.dma_start(w2t[:], w2_d[:])
                # prologue
                mm1_chunk_pe(0, defer_bias=False)
                p1_build(0)
                nc.vector.memset(v1_3[:, :, 0], 0.0)
                nc.vector.tensor_copy(za[:], p1_3[:, :, 0])

                # phase c: PE runs mm2 chunk c-1 then mm1 chunk c+1 while
                # DVE/Act run scan1 seg c interleaved with scan2 seg c-1.
                # Slow-engine tails (st2 bias+DMA, qb bias) are sprinkled
                # into the scan stream so no engine queue stalls behind an
                # unready op.
                PREFIX = 32  # scan1-only steps before p2-build(c-1)
                for c in range(NTC + 1):
                    st2_extras = list(mm2_mm(c - 1)) if 1 <= c <= NTC else []
                    qb_extras = (mm1_chunk_pe(c + 1)
                                 if c + 1 <= NTC - 1 else [])
                    s1 = range(*seg[c]) if c < NTC else range(0)
                    s2 = range(*seg[c - 1]) if c >= 1 else range(0)
                    it1, it2 = iter(s1), iter(s2)
                    if c >= 1:
                        # scan1-only prefix; st2 tails every 3rd step
                        for k in range(PREFIX):
                            t1 = next(it1, None)
                            if t1 is not None:
                                scan1_step(t1)
                            if k % 3 == 2 and st2_extras:
                                st2_extras.pop(0)()
                        while st2_extras:
                            st2_extras.pop(0)()
                        p2_build(c - 1)
                        if c == 1:
                            nc.vector.memset(v2[:, 0:1], 0.0)
                            nc.vector.tensor_copy(za2[:], p2[:, 0:1])
                    k = 0
                    while True:
                        t1 = next(it1, None)
                        if t1 is not None:
                            scan1_step(t1)
                        t2_ = next(it2, None)
                        if t2_ is not None:
                            scan2_step(t2_)
                        if k % 16 == 15 and qb_extras:
                            qb_extras.pop(0)()
                        k += 1
                        if t1 is None and t2_ is None:
                            break
                    while qb_extras:
                        qb_extras.pop(0)()
                    if c + 1 <= NTC - 1:
                        p1_build(c + 1)

                # unscale into q2 (dead) and DMA out
                nc.vector.tensor_scalar(q2[:, 0:T], v2[:], 1.0 / K_SC, None,
                                        AOp.mult)
                for b in range(BL):
                    nc.sync.dma_start(
                        out_d[b].rearrange("t m -> m t"),
                        q2[b * M:(b + 1) * M, 0:T])

            for _ in range(reps):
                emit_once()

    return nc


def _build_kernel_v4(scan_steps=None, reps=1):
    """Merged-scan variant: scan2's 80 state elements live on partitions
    100-109 (10 neurons x 8 sample-slots), time-shifted +512 columns, so ONE
    set of 5 all-DVE ops per combined step advances BOTH scans ([110, 8]
    operands, no cross-engine hop in the recurrence chain):

        u  = V*V                     (tensor_tensor)
        t2 = alpha*Z - V             (scalar_tensor_tensor)
        Z' = t2 + p[col t+1]         (tensor_tensor)
        m  = (u - A)*V               (scalar_tensor_tensor)
        V' = beta*Z - m              (scalar_tensor_tensor)

    Layouts (sample/slot-major): rows 0-99 of pp/vv hold p1/V1 at col
    b*TP|b*T + t; rows 100-109 hold p2/V2 for slot b at col b*TP|b*T +
    tau + 512 (tau = scan2 time = t - 512). Slot b's tail columns overlap
    slot b+1's head-garbage region; garbage is written before real data
    arrives and never read for output. A 512-step tail (rows 100-109 only)
    finishes scan2 after scan1 ends. mm1/mm2/p-builds keep the v3 chunk
    pipeline; PE/Act/DMA tails are sprinkled into the scan stream."""
    assert scan_steps is None
    import concourse.bass as bass
    import concourse.tile as tile
    from concourse import mybir

    f32 = mybir.dt.float32
    AOp = mybir.AluOpType

    nc = bass.Bass()
    _cb = nc.alloc_sbuf_tensor("const-cbias", [128, 1], f32)
    nc.gpsimd.memset(_cb.ap(), -C_BIAS)
    nc.const_aps.aps[(f32, -C_BIAS)] = _cb.ap()
    nc.all_engine_barrier()

    bt_d = nc.declare_dram_parameter("batchT", [BL, D, T], f32, isOutput=False)
    w1_d = nc.declare_dram_parameter("W1T", [D, N], f32, isOutput=False)
    w2_d = nc.declare_dram_parameter("W2T", [N, M], f32, isOutput=False)
    out_d = nc.declare_dram_parameter("out", [BL, T, M], f32, isOutput=True)

    NR = N + M          # 110 combined rows
    SH = TCH + 64       # scan2 time shift: one chunk + p2-build prefix slack

    with tile.TileContext(nc) as tc:
        with (
            tc.tile_pool(name="const", bufs=1) as cpool,
            tc.tile_pool(name="bt", bufs=4) as btpool,
            tc.tile_pool(name="qs", bufs=8) as qspool,
            tc.tile_pool(name="ps1", bufs=4, space="PSUM") as ps1pool,
            tc.tile_pool(name="ps2", bufs=4, space="PSUM") as ps2pool,
            tc.tile_pool(name="big", bufs=1) as bigpool,
            tc.tile_pool(name="small", bufs=6) as spool,
            tc.tile_pool(name="st2p", bufs=8) as st2pool,
            tc.tile_pool(name="p2cp", bufs=3) as p2cpool,
            tc.tile_pool(name="state", bufs=2) as stpool,
        ):
            w1t = cpool.tile([KCH, KC * N], f32)
            w2t = cpool.tile([N, M], f32)
            lastq = cpool.tile([N, BL], f32)

            # one extra slot of width so tail columns (slot b spilling into
            # slot b+1's range, and slot 7 into the pad) stay in-bounds
            pp = bigpool.tile([NR, (BL + 1) * TP], f32)   # p1 | p2 (shifted)
            vv = bigpool.tile([NR, (BL + 1) * T], f32)    # V1 | V2 (shifted)
            vout = bigpool.tile([NR, T], f32)             # unscaled V2 staging

            za = stpool.tile([NR, BL], f32, tag="za")
            zb = stpool.tile([NR, BL], f32, tag="zb")

            # ---------------- emission helpers -----------------------------
            qbs = {}

            def mm1_chunk_pe(c, defer_bias=True):
                closures = []
                for b in range(BL):
                    ps = ps1pool.tile([N, TCH], f32)
                    for i in range(KC):
                        bt = btpool.tile([KCH, TCH], f32)
                        nc.sync.dma_start(
                            bt[:],
                            bt_d[b, i * KCH:(i + 1) * KCH,
                                 c * TCH:(c + 1) * TCH],
                        )
                        nc.tensor.matmul(
                            ps[:], lhsT=w1t[:, i * N:(i + 1) * N], rhs=bt[:],
                            start=(i == 0), stop=(i == KC - 1),
                        )

                    def bias(b=b, ps=ps):
                        qb = qspool.tile([N, TCH], f32, tag="qhat")
                        nc.scalar.add(qb[:], ps[:], -C_BIAS)
                        qbs[(b, c)] = qb
                    if defer_bias:
                        closures.append(bias)
                    else:
                        bias()
                return closures

            def p1_build(c):
                for b in range(BL):
                    qb = qbs.pop((b, c))
                    s = pp[0:N, b * TP:(b + 1) * TP]
                    t0 = c * TCH
                    if c == 0:
                        nc.vector.tensor_scalar(
                            s[:, 0:1], qb[:, 0:1], C_BIAS, None, AOp.add)
                    else:
                        nc.vector.scalar_tensor_tensor(
                            s[:, t0:t0 + 1], lastq[:, b:b + 1], -ALPHA,
                            qb[:, 0:1], AOp.mult, AOp.add)
                    nc.vector.scalar_tensor_tensor(
                        s[:, t0 + 1:t0 + TCH], qb[:, 0:TCH - 1], -ALPHA,
                        qb[:, 1:TCH], AOp.mult, AOp.add)
                    if c < NTC - 1:
                        nc.vector.tensor_copy(
                            lastq[:, b:b + 1], qb[:, TCH - 1:TCH])

            st2s = {}
            lastq2 = cpool.tile([M, BL], f32)

            def mm2_mm(c):
                """PE mm2 chunk c -> PSUM rows 0-9; Act bias-adds returned
                as closures (st2 on rows 0-9, start-partition 0)."""
                closures = []
                for b in range(BL):
                    ps2 = ps2pool.tile([M, TCH], f32)
                    nc.tensor.matmul(
                        ps2[:], lhsT=w2t[:],
                        rhs=vv[0:N, b * T + c * TCH: b * T + (c + 1) * TCH],
                        start=True, stop=True)

                    def tail(b=b, ps2=ps2):
                        st2 = st2pool.tile([M, TCH], f32, tag="q2st")
                        nc.scalar.add(st2[:], ps2[:], -C_BIAS)
                        st2s[(b, c)] = st2
                    closures.append(tail)
                return closures

            def p2_build(c):
                """Build p2 chunk c on rows 0-9, then DMA it to pp rows
                100-109 at cols b*TP + SH + tau (DVE/Act cannot address a
                start partition of 100; DMA can)."""
                t0 = c * TCH
                for b in range(BL):
                    st2 = st2s.pop((b, c))
                    q = st2[:]
                    p2c = p2cpool.tile([M, TCH], f32, tag="p2c")
                    if c == 0:
                        nc.vector.tensor_scalar(
                            p2c[:, 0:1], q[:, 0:1], C_BIAS, None, AOp.add)
                    else:
                        nc.vector.scalar_tensor_tensor(
                            p2c[:, 0:1], lastq2[:, b:b + 1], -ALPHA,
                            q[:, 0:1], AOp.mult, AOp.add)
                    nc.vector.scalar_tensor_tensor(
                        p2c[:, 1:TCH], q[:, 0:TCH - 1], -ALPHA, q[:, 1:TCH],
                        AOp.mult, AOp.add)
                    if c < NTC - 1:
                        nc.vector.tensor_copy(
                            lastq2[:, b:b + 1], q[:, TCH - 1:TCH])
                    nc.sync.dma_start(
                        pp[N:NR, b * TP + SH + t0:b * TP + SH + t0 + TCH],
                        p2c[:])

            pp3 = pp[:].rearrange("p (b t) -> p b t", b=BL + 1)
            vv3 = vv[:].rearrange("p (b t) -> p b t", b=BL + 1)

            def vcol(rows, t):
                if t < T:
                    return vv3[rows, 0:BL, t]
                return vv3[rows, 1:BL + 1, t - T]

            def pcol(rows, t):
                if t < TP:
                    return pp3[rows, 0:BL, t]
                return pp3[rows, 1:BL + 1, t - TP]

            def step(t, rows=slice(0, NR)):
                """One combined scan step (all-DVE, 5 ops)."""
                zs, zn = (za, zb) if t % 2 == 0 else (zb, za)
                vt = vcol(rows, t)
                vn = vcol(rows, t + 1)
                pc = pcol(rows, t + 1)
                zs = zs[rows, :]
                zn = zn[rows, :]
                u_t = spool.tile([NR, BL], f32, tag="u")
                u = u_t[rows, :]
                nc.vector.tensor_tensor(u, vt, vt, AOp.mult)
                t2_t = spool.tile([NR, BL], f32, tag="t2")
                t2 = t2_t[rows, :]
                nc.vector.scalar_tensor_tensor(
                    t2, zs, ALPHA, vt, AOp.mult, AOp.subtract)
                nc.vector.tensor_tensor(zn, t2, pc, AOp.add)
                m_t = spool.tile([NR, BL], f32, tag="m")
                m = m_t[rows, :]
                nc.vector.scalar_tensor_tensor(
                    m, u, A_CONST, vt, AOp.subtract, AOp.mult)
                nc.vector.scalar_tensor_tensor(
                    vn, zs, BETA, m, AOp.mult, AOp.subtract)

            P0 = 96   # lowest legal start partition covering rows 100-109
            sav1 = cpool.tile([NR, BL], f32)
            sav2 = cpool.tile([NR, BL], f32)

            def bootstrap2():
                """Reset rows 100-109 state for scan2 start (combined t=SH).
                Runs between step(SH-1) and step(SH); za holds Z(SH). Engine
                APs must start at partition 96, so the live scan1 rows 96-99
                are saved and restored around the wide writes."""
                nc.vector.tensor_copy(sav1[P0:N, :], vv3[P0:N, 0:BL, SH])
                nc.vector.memset(vv3[P0:NR, 0:BL, SH], 0.0)
                nc.vector.tensor_copy(vv3[P0:N, 0:BL, SH], sav1[P0:N, :])
                nc.vector.tensor_copy(sav2[P0:N, :], za[P0:N, :])
                nc.vector.tensor_copy(za[P0:NR, :], pp3[P0:NR, 0:BL, SH])
                nc.vector.tensor_copy(za[P0:N, :], sav2[P0:N, :])

            # seg[c]: combined steps of phase c (scan1 rows produce V1 cols
            # [c*TCH, (c+1)*TCH); scan2 rows trail by SH columns)
            seg = []
            start = 0
            for c in range(NTC):
                end = (c + 1) * TCH - 1 if c < NTC - 1 else T - 1
                seg.append((start, end))
                start = end

            PREF2 = 56   # steps into a phase before p2_build(c-1); must be
            # < SH - TCH - 3 so every p2 chunk lands before its first read

            def emit_once():
                for i in range(KC):
                    nc.sync.dma_start(
                        w1t[:, i * N:(i + 1) * N],
                        w1_d[i * KCH:(i + 1) * KCH, :])
                nc.sync.dma_start(w2t[:], w2_d[:])
                # head-garbage regions read by rows 100-109 before bootstrap
                # (start partition 96; rows 96-99 are rebuilt afterwards)
                nc.gpsimd.memset(pp[P0:NR, :], 0.0)
                nc.gpsimd.memset(vv[P0:NR, :], 0.0)
                mm1_chunk_pe(0, defer_bias=False)
                p1_build(0)
                nc.vector.memset(vv3[0:N, 0:BL, 0], 0.0)
                nc.vector.tensor_copy(za[:], pp3[:, 0:BL, 0])

                for c in range(NTC):
                    st2_extras = list(mm2_mm(c - 1)) if c >= 1 else []
                    qb_extras = (mm1_chunk_pe(c + 1)
                                 if c + 1 <= NTC - 1 else [])
                    k = 0
                    for t in range(*seg[c]):
                        if t == SH:
                            bootstrap2()
                        step(t)
                        if k % 3 == 2 and st2_extras:
                            st2_extras.pop(0)()
                        if k == PREF2 and c >= 1:
                            while st2_extras:
                                st2_extras.pop(0)()
                            p2_build(c - 1)
                        if k % 16 == 15 and qb_extras:
                            qb_extras.pop(0)()
                        k += 1
                    while st2_extras:
                        st2_extras.pop(0)()
                    while qb_extras:
                        qb_extras.pop(0)()
                    if c + 1 <= NTC - 1:
                        p1_build(c + 1)

                # tail: scan2 rows only, combined t in [T-1, T-1+SH)
                st2_extras = list(mm2_mm(NTC - 1))
                k = 0
                for t in range(T - 1, T - 1 + SH):
                    step(t, rows=slice(P0, NR))
                    if k % 3 == 2 and st2_extras:
                        st2_extras.pop(0)()
                    if k == PREF2:
                        while st2_extras:
                            st2_extras.pop(0)()
                        p2_build(NTC - 1)
                    k += 1

                # unscale + transposed DMA out, per sample slot
                for b in range(BL):
                    nc.vector.tensor_scalar(
                        vout[P0:NR, :], vv[P0:NR, b * T + SH: b * T + SH + T],
                        1.0 / K_SC, None, AOp.mult)
                    nc.sync.dma_start(
                        out_d[b].rearrange("t m -> m t"), vout[N:NR, :])

            for _ in range(reps):
                emit_once()

    return nc


def _build_kernel(scan_steps=None):
    import concourse.bass as bass
    import concourse.tile as tile
    from concourse import mybir

    f32 = mybir.dt.float32
    AOp = mybir.AluOpType

    nc = bass.Bass()
    # register the bias constant for ScalarE add (activation bias const-AP)
    _cb = nc.alloc_sbuf_tensor("const-cbias", [128, 1], f32)
    nc.gpsimd.memset(_cb.ap(), -C_BIAS)
    nc.const_aps.aps[(f32, -C_BIAS)] = _cb.ap()
    nc.all_engine_barrier()

    bt_d = nc.declare_dram_parameter("batchT", [BL, D, T], f32, isOutput=False)
    w1_d = nc.declare_dram_parameter("W1T", [D, N], f32, isOutput=False)
    w2_d = nc.declare_dram_parameter("W2T", [N, M], f32, isOutput=False)
    out_d = nc.declare_dram_parameter("out", [BL, T, M], f32, isOutput=True)

    with tile.TileContext(nc) as tc:
        with (
            tc.tile_pool(name="const", bufs=1) as cpool,
            tc.tile_pool(name="bt", bufs=3) as btpool,
            tc.tile_pool(name="qs", bufs=3) as qspool,
            tc.tile_pool(name="ps1", bufs=4, space="PSUM") as ps1pool,
            tc.tile_pool(name="ps2", bufs=2, space="PSUM") as ps2pool,
            tc.tile_pool(name="big", bufs=1) as bigpool,
            tc.tile_pool(name="small", bufs=3) as spool,
            tc.tile_pool(name="state", bufs=2) as stpool,
        ):
            w1t = cpool.tile([KCH, KC * N], f32)
            for i in range(KC):
                nc.sync.dma_start(
                    w1t[:, i * N:(i + 1) * N], w1_d[i * KCH:(i + 1) * KCH, :]
                )
            w2t = cpool.tile([N, M], f32)
            nc.sync.dma_start(w2t[:], w2_d[:])

            p1 = bigpool.tile([N, BL * TP], f32)    # scan-1 p-stream, per sample
            v1 = bigpool.tile([N, BL * T], f32)     # scan-1 output (Vt1)
            q2 = bigpool.tile([BL * M, T], f32)     # q-hat-2, reused as Vt2
            p2 = bigpool.tile([BL * M, TP], f32)    # scan-2 p-stream

            p1_3 = p1[:].rearrange("p (b t) -> p b t", b=BL)
            v1_3 = v1[:].rearrange("p (b t) -> p b t", b=BL)

            # ---------------- mm1 + per-sample stream build ------------------
            for b in range(BL):
                qb = qspool.tile([N, T], f32, tag="qhat")
                for c in range(NTC):
                    ps = ps1pool.tile([N, TCH], f32)
                    for i in range(KC):
                        bt = btpool.tile([KCH, TCH], f32)
                        nc.sync.dma_start(
                            bt[:],
                            bt_d[b, i * KCH:(i + 1) * KCH,
                                 c * TCH:(c + 1) * TCH],
                        )
                        nc.tensor.matmul(
                            ps[:], lhsT=w1t[:, i * N:(i + 1) * N], rhs=bt[:],
                            start=(i == 0), stop=(i == KC - 1),
                        )
                    nc.scalar.add(qb[:, c * TCH:(c + 1) * TCH], ps[:], -C_BIAS)
                s = p1[:, b * TP:(b + 1) * TP]
                nc.vector.tensor_scalar(
                    s[:, 0:1], qb[:, 0:1], C_BIAS, None, AOp.add)
                nc.vector.scalar_tensor_tensor(
                    s[:, 1:T], qb[:, 0:T - 1], -ALPHA, qb[:, 1:T],
                    AOp.mult, AOp.add)
                nc.gpsimd.memset(s[:, T:T + 1], 0.0)

            # ---------------- scan 1 (all samples per instruction) ----------
            # state cols: V_t lives in v1[:, :, t]; Z in ping-pong tiles.
            nc.vector.memset(v1_3[:, :, 0], 0.0)
            za = stpool.tile([N, BL], f32, tag="za")
            zb = stpool.tile([N, BL], f32, tag="zb")
            # Z_0 = qhat_0 + c  (= p-stream col 0 + ... col0 holds qhat0 + c)
            nc.vector.tensor_copy(za[:], p1_3[:, :, 0])
            SQ = mybir.ActivationFunctionType.Square
            _S1 = (T - 1) if scan_steps is None else scan_steps
            for t in range(_S1):
                zs, zn = (za, zb) if t % 2 == 0 else (zb, za)
                vt = v1_3[:, :, t]
                u = spool.tile([N, BL], f32, tag="u")
                nc.scalar.activation(u[:], vt, SQ)        # u = V^2 (ScalarE)
                m = spool.tile([N, BL], f32, tag="m")
                # m' = (u - A) * V  == -(A - u)*V
                nc.vector.scalar_tensor_tensor(
                    m[:], u[:], A_CONST, vt, AOp.subtract, AOp.mult)
                # V' = beta*Z - m'
                nc.vector.scalar_tensor_tensor(
                    v1_3[:, :, t + 1], zs[:], BETA, m[:], AOp.mult,
                    AOp.subtract)
                t2 = spool.tile([N, BL], f32, tag="t2")
                nc.vector.scalar_tensor_tensor(
                    t2[:], zs[:], ALPHA, vt, AOp.mult, AOp.subtract)
                nc.vector.tensor_tensor(zn[:], t2[:], p1_3[:, :, t + 1],
                                        AOp.add)

            # ---------------- mm2 ------------------------------------------
            for b in range(BL):
                for c in range(NTC):
                    ps2 = ps2pool.tile([M, TCH], f32)
                    nc.tensor.matmul(
                        ps2[:], lhsT=w2t[:],
                        rhs=v1[:, b * T + c * TCH: b * T + (c + 1) * TCH],
                        start=True, stop=True)
                    st2 = spool.tile([M, TCH], f32, tag="q2st")
                    nc.scalar.add(st2[:], ps2[:], -C_BIAS)
                    nc.sync.dma_start(
                        q2[b * M:(b + 1) * M, c * TCH:(c + 1) * TCH], st2[:])

            # ---------------- scan 2 ([80, 1] slices) -----------------------
            nc.vector.tensor_scalar(
                p2[:, 0:1], q2[:, 0:1], C_BIAS, None, AOp.add)
            nc.vector.scalar_tensor_tensor(
                p2[:, 1:T], q2[:, 0:T - 1], -ALPHA, q2[:, 1:T],
                AOp.mult, AOp.add)
            nc.gpsimd.memset(p2[:, T:T + 1], 0.0)

            P2 = BL * M
            za2 = stpool.tile([P2, 1], f32, tag="za2")
            zb2 = stpool.tile([P2, 1], f32, tag="zb2")
            nc.vector.tensor_copy(za2[:], p2[:, 0:1])
            nc.vector.memset(q2[:, 0:1], 0.0)   # V2 col 0 (q2 reused as V2)
            _S2 = (T - 1) if scan_steps is None else scan_steps
            for t in range(_S2):
                zs, zn = (za2, zb2) if t % 2 == 0 else (zb2, za2)
                vt = q2[:, t:t + 1]
                u = spool.tile([P2, 1], f32, tag="u2")
                nc.vector.tensor_tensor(u[:], vt, vt, AOp.mult)
                r = spool.tile([P2, 1], f32, tag="r2")
                nc.vector.tensor_scalar(r[:], u[:], -1.0, A_CONST,
                                        AOp.mult, AOp.add)
                m = spool.tile([P2, 1], f32, tag="m2")
                nc.vector.tensor_tensor(m[:], vt, r[:], AOp.mult)
                nc.vector.scalar_tensor_tensor(
                    q2[:, t + 1:t + 2], zs[:], BETA, m[:], AOp.mult, AOp.add)
                t2 = spool.tile([P2, 1], f32, tag="t22")
                nc.vector.scalar_tensor_tensor(
                    t2[:], zs[:], ALPHA, vt, AOp.mult, AOp.subtract)
                nc.vector.tensor_tensor(zn[:], t2[:], p2[:, t + 1:t + 2],
                                        AOp.add)

            # unscale into p2 (dead) and DMA out
            nc.vector.tensor_scalar(p2[:, 0:T], q2[:], 1.0 / K_SC, None,
                                    AOp.mult)
            for b in range(BL):
                nc.sync.dma_start(
                    out_d[b].rearrange("t m -> m t"),
                    p2[b * M:(b + 1) * M, 0:T])

    return nc


def kernel(batch, W1, W2):
    _install_bir_patch()
    from concourse.bass_utils import run_bass_kernel_spmd

    if "nc" not in _CACHE:
        _CACHE["nc"] = _build_kernel_v3()
    nc = _CACHE["nc"]

    batch = np.asarray(batch, dtype=np.float32)
    W1 = np.asarray(W1, dtype=np.float32)
    W2 = np.asarray(W2, dtype=np.float32)

    w1t = np.ascontiguousarray((KOB * DT * W1).T.astype(np.float32))
    w2t = np.ascontiguousarray(((DT * 0.5 / BETA) * W2).T.astype(np.float32))

    in_maps = []
    for cidx in range(NCORES):
        sl = batch[cidx * BL:(cidx + 1) * BL]
        btT = np.ascontiguousarray(sl.transpose(0, 2, 1))
        in_maps.append({"batchT": btT, "W1T": w1t, "W2T": w2t})

    res = run_bass_kernel_spmd(nc, in_maps, list(range(NCORES)))
    out = np.concatenate([res.results[i]["out"] for i in range(NCORES)], axis=0)
    return out.astype(np.float32)

